# revision 10
# baseline (speedup 1.0000x reference)
"""Trainium2 Bass kernel for multi-head causal attention with RoPE.

Problem: B=4, T=2048, D=2048, H=16 heads (HD=128), fp32 reference:
  q/k/v = x @ w{q,k,v}.T ; RoPE(q,k) ; causal softmax(q k^T/sqrt(HD)) @ v ; @ wo.T

Sharding over 8 cores: 4 batch shards x 2 head-groups (8 heads each).
Each core: projections for its heads (column-split weights), attention,
chunked AllGather of attention outputs within the batch pair (overlapped with
attention compute), then output-column-split wo matmul (each core produces
out[:, g*1024:(g+1)*1024] for its batch).

Matmuls run in bf16 (PE at 1 cycle/row); accumulation + softmax in fp32.
Attention computes S^T = k^T q per block so no probs transpose is needed;
softmax row-sums come from a ones-matmul (broadcast across partitions) and
normalization is folded into the PSUM eviction multiply.
"""

import math
import sys
from contextlib import ExitStack

sys.path.insert(0, "/opt/trn_rl_repo")

import numpy as np
import ml_dtypes

import concourse.bass as bass
import concourse.mybir as mybir
import concourse.tile as tile
from concourse import bacc
from concourse.bass_utils import run_bass_kernel_spmd

BF16 = ml_dtypes.bfloat16
B, T, D, H, HD = 4, 2048, 2048, 16, 128
HL = 8            # heads per core
DL = HL * HD      # local feature width (1024)
P = 128
NB = 512          # free-dim block for matmuls
N_CORES = 8
N_CHUNK = 4       # gather chunks (2 heads each)
INV_SQRT_HD = 1.0 / math.sqrt(HD)

dt = mybir.dt
f32 = dt.float32
bf16 = dt.bfloat16


def build_program(t=T, n_cores=N_CORES, with_collective="tail"):
    """Build + compile the per-core Bass program (SPMD, identical on all cores)."""
    n_tb = t // NB      # 512-wide token blocks
    n_tt = t // P       # 128-wide token tiles
    n_db = D // P       # contraction blocks over model dim
    n_q = t // NB       # query blocks (512)

    mult = mybir.AluOpType.mult
    addop = mybir.AluOpType.add
    Exp = mybir.ActivationFunctionType.Exp

    nc = bacc.Bacc("TRN2", target_bir_lowering=False, debug=False,
                   num_devices=n_cores)

    xT = nc.dram_tensor("xT", [D, t], bf16, kind="ExternalInput").ap()
    wqT = nc.dram_tensor("wqT", [D, DL], bf16, kind="ExternalInput").ap()
    wkT = nc.dram_tensor("wkT", [D, DL], bf16, kind="ExternalInput").ap()
    wvT = nc.dram_tensor("wvT", [D, DL], bf16, kind="ExternalInput").ap()
    woT = nc.dram_tensor("woT", [D, DL], bf16, kind="ExternalInput").ap()
    cosh = nc.dram_tensor("cosh", [P, t], f32, kind="ExternalInput").ap()
    sinh = nc.dram_tensor("sinh", [P, t], f32, kind="ExternalInput").ap()
    trimulT = nc.dram_tensor("trimulT", [P, P], bf16, kind="ExternalInput").ap()
    out_part = nc.dram_tensor("out_part", [t, DL], f32, kind="ExternalOutput").ap()

    heads_per_chunk = HL // N_CHUNK

    with tile.TileContext(nc) as tc:
        with tc.tile_pool(name="dram", bufs=1, space="DRAM") as dram:
            attnLc = [dram.tile([heads_per_chunk * P, t], bf16, name=f"attnL{c}")
                      for c in range(N_CHUNK)]
            attnFc = [dram.tile([2 * heads_per_chunk * P, t], bf16,
                                name=f"attnF{c}")
                      for c in range(N_CHUNK)]

            with ExitStack() as es:
                persist = es.enter_context(tc.tile_pool(name="persist", bufs=1))
                xpool_cm = tc.tile_pool(name="xpool", bufs=2)
                xpool = xpool_cm.__enter__()
                qT = [persist.tile([P, t], bf16, tag=f"qT{h}", name=f"qT{h}")
                      for h in range(HL)]
                kT = [persist.tile([P, t], bf16, tag=f"kT{h}", name=f"kT{h}")
                      for h in range(HL)]
                tri_sb = persist.tile([P, P], bf16, tag="tri")
                ones_sb = persist.tile([P, P], bf16, tag="ones")
                nc.vector.memset(ones_sb[:], 1.0)

                # ============ Phase 1a: Q+K projections (share x tiles) ======
                with (
                    tc.tile_pool(name="ropec", bufs=1) as ropec,
                    tc.tile_pool(name="wpool", bufs=1) as wpool,
                    tc.tile_pool(name="pj_psum", bufs=4, space="PSUM") as pjp,
                    tc.tile_pool(name="rope_tmp", bufs=3) as rtmp,
                ):
                    wq_t = [wpool.tile([P, DL], bf16, tag=f"wq{db}",
                                       name=f"wq{db}") for db in range(n_db)]
                    wk_t = [wpool.tile([P, DL], bf16, tag=f"wk{db}",
                                       name=f"wk{db}") for db in range(n_db)]
                    cos_sb = ropec.tile([P, t], f32, tag="cos")
                    sin_sb = ropec.tile([P, t], f32, tag="sin")
                    for db in range(n_db):
                        nc.scalar.dma_start(wq_t[db][:], wqT[db * P:(db + 1) * P, :])

                    def rope_evict(ps, dst_tile, tsl):
                        u = rtmp.tile([P, NB], f32, tag="u", name="u")
                        nc.vector.tensor_tensor(u[0:64, :], ps[64:128, :],
                                                sin_sb[0:64, tsl], op=mult)
                        nc.vector.tensor_tensor(u[64:128, :], ps[0:64, :],
                                                sin_sb[64:128, tsl], op=mult)
                        nc.vector.tensor_tensor(dst_tile, ps[:], cos_sb[:, tsl],
                                                op=mult)
                        nc.vector.tensor_tensor(dst_tile, dst_tile, u[:], op=addop)

                    for tb in range(n_tb):
                        tsl = bass.ts(tb, NB)
                        x_t = [xpool.tile([P, NB], bf16, tag=f"x{db}",
                                          name=f"x{db}") for db in range(n_db)]
                        for db in range(n_db):
                            nc.sync.dma_start(x_t[db][:],
                                              xT[db * P:(db + 1) * P, tsl])
                        if tb == 0:
                            # emit late-needed loads behind the critical path
                            for db in range(n_db):
                                nc.scalar.dma_start(wk_t[db][:],
                                                    wkT[db * P:(db + 1) * P, :])
                            nc.scalar.dma_start(cos_sb[:], cosh[:])
                            nc.scalar.dma_start(sin_sb[:], sinh[:])
                            nc.scalar.dma_start(tri_sb[:], trimulT[:])
                        for jt in range(HL):
                            for (w_t, dst) in ((wq_t, qT), (wk_t, kT)):
                                ps = pjp.tile([P, NB], f32, name="ps")
                                for db in range(n_db):
                                    nc.tensor.matmul(
                                        ps[:], lhsT=w_t[db][:, bass.ts(jt, P)],
                                        rhs=x_t[db][:],
                                        start=(db == 0), stop=(db == n_db - 1))
                                rope_evict(ps, dst[jt][:, tsl], tsl)

                # ============ Phase 1b: V projection =========================
                # v tiles live in their own pool, reusing the closed wpool space
                vpool = es.enter_context(
                    tc.tile_pool(name="vpool", bufs=1, side="right"))
                v = [vpool.tile([P, DL], bf16, tag=f"v{tt}", name=f"v{tt}")
                     for tt in range(n_tt)]
                with (
                    tc.tile_pool(name="wvpool", bufs=1) as wvpool,
                    tc.tile_pool(name="v_psum", bufs=4, space="PSUM") as vps,
                ):
                    wv_t = [wvpool.tile([P, DL], bf16, tag=f"wv{db}",
                                        name=f"wv{db}") for db in range(n_db)]
                    for db in range(n_db):
                        nc.scalar.dma_start(wv_t[db][:], wvT[db * P:(db + 1) * P, :])
                    for tb in range(n_tb):
                        x_t = [xpool.tile([P, NB], bf16, tag=f"x{db}",
                                          name=f"x{db}") for db in range(n_db)]
                        for db in range(n_db):
                            nc.sync.dma_start(
                                x_t[db][:], xT[db * P:(db + 1) * P, bass.ts(tb, NB)])
                        for tq in range(4):
                            tt = tb * 4 + tq
                            for jb in range(DL // NB):
                                ps = vps.tile([P, NB], f32, name="ps")
                                for db in range(n_db):
                                    nc.tensor.matmul(
                                        ps[:], lhsT=x_t[db][:, bass.ts(tq, P)],
                                        rhs=wv_t[db][:, bass.ts(jb, NB)],
                                        start=(db == 0), stop=(db == n_db - 1))
                                nc.vector.tensor_copy(v[tt][:, bass.ts(jb, NB)],
                                                      ps[:])
                xpool_cm.__exit__(None, None, None)

                # ============ Phase 2: attention (+ chunked gather) ==========
                with (
                    tc.tile_pool(name="st_psum", bufs=3, space="PSUM") as stp,
                    tc.tile_pool(name="ot_psum", bufs=2, space="PSUM") as otp,
                    tc.tile_pool(name="sum_psum", bufs=2, space="PSUM") as smp,
                    tc.tile_pool(name="ptu", bufs=6) as ptup,
                    tc.tile_pool(name="att_ev", bufs=4) as atev,
                ):
                    for h in range(HL):
                        for q in range(n_q):
                            qsl = bass.ts(q, NB)
                            nkb = 4 * q + 4
                            ot = otp.tile([P, NB], f32, tag="ot", name="ot")
                            sums = smp.tile([P, NB], f32, tag="sums", name="sums")
                            for kb in range(nkb):
                                rel = kb - 4 * q
                                st = stp.tile([P, NB], f32, tag="st", name="st")
                                nc.tensor.matmul(
                                    st[:], lhsT=kT[h][:, bass.ts(kb, P)],
                                    rhs=qT[h][:, qsl], start=True, stop=True)
                                ptu = ptup.tile([P, NB], bf16, tag="ptu",
                                                name="ptu")
                                if rel >= 0:
                                    if rel > 0:
                                        nc.gpsimd.memset(ptu[:, 0:rel * P], 0.0)
                                    esl = bass.ds(rel * P, NB - rel * P)
                                    nc.scalar.activation(ptu[:, esl], st[:, esl],
                                                         Exp, scale=INV_SQRT_HD)
                                    csl = bass.ds(rel * P, P)
                                    nc.vector.tensor_tensor(
                                        ptu[:, csl], ptu[:, csl], tri_sb[:],
                                        op=mult)
                                else:
                                    nc.scalar.activation(ptu[:], st[:],
                                                         Exp, scale=INV_SQRT_HD)
                                nc.tensor.matmul(
                                    sums[:], lhsT=ones_sb[:], rhs=ptu[:],
                                    start=(kb == 0), stop=(kb == nkb - 1))
                                nc.tensor.matmul(
                                    ot[:], lhsT=v[kb][:, bass.ts(h, P)], rhs=ptu[:],
                                    start=(kb == 0), stop=(kb == nkb - 1))
                            # normalize with 1/sums (already partition-broadcast)
                            rb = atev.tile([P, NB], f32, tag="rb", name="rb")
                            nc.vector.reciprocal_approx_fast(out=rb[:], in_=sums[:])
                            att = atev.tile([P, NB], bf16, tag="att", name="att")
                            nc.vector.tensor_tensor(att[:], ot[:], rb[:], op=mult)
                            c = h // heads_per_chunk
                            row = (h % heads_per_chunk) * P
                            nc.sync.dma_start(attnLc[c][row:row + P, qsl], att[:])
                        if (with_collective == "inline"
                                and h % heads_per_chunk == heads_per_chunk - 1):
                            c = h // heads_per_chunk
                            nc.gpsimd.collective_compute(
                                "AllGather", mybir.AluOpType.bypass,
                                replica_groups=[[i, i + 1]
                                                for i in range(0, n_cores, 2)],
                                ins=[attnLc[c].opt()], outs=[attnFc[c].opt()],
                            )

            if with_collective == "tail":
                for c in range(N_CHUNK):
                    nc.gpsimd.collective_compute(
                        "AllGather", mybir.AluOpType.bypass,
                        replica_groups=[[i, i + 1]
                                        for i in range(0, n_cores, 2)],
                        ins=[attnLc[c].opt()], outs=[attnFc[c].opt()],
                    )
            elif not with_collective:
                for c in range(N_CHUNK):
                    hp = heads_per_chunk * P
                    nc.sync.dma_start(attnFc[c][0:hp, :], attnLc[c][:])
                    nc.sync.dma_start(attnFc[c][hp:2 * hp, :], attnLc[c][:])

            # ============ Phase 3: wo matmul =============================
            n_fb = (2 * DL) // P
            fb_per_chunk = n_fb // N_CHUNK
            with (
                tc.tile_pool(name="afpool", bufs=1) as afp,
                tc.tile_pool(name="wopool", bufs=1) as wop,
                tc.tile_pool(name="accpool", bufs=1) as accp,
                tc.tile_pool(name="wo_psum", bufs=4, space="PSUM") as wps,
                tc.tile_pool(name="out_ev", bufs=4) as oev,
            ):
                af_t = [afp.tile([P, t], bf16, tag=f"af{fb}", name=f"af{fb}")
                        for fb in range(n_fb)]
                for fb in range(n_fb):
                    c, r = fb // fb_per_chunk, fb % fb_per_chunk
                    nc.scalar.dma_start(af_t[fb][:],
                                        attnFc[c][r * P:(r + 1) * P, :])
                wo_t = [wop.tile([P, DL], bf16, tag=f"wo{fb}", name=f"wo{fb}")
                        for fb in range(n_fb)]
                for fb in range(n_fb):
                    nc.sync.dma_start(wo_t[fb][:], woT[fb * P:(fb + 1) * P, :])
                acc = [accp.tile([P, DL], f32, tag=f"acc{tt}", name=f"acc{tt}")
                       for tt in range(n_tt)]
                addop2 = mybir.AluOpType.add
                for c in range(N_CHUNK):
                    fb0 = c * fb_per_chunk
                    for tt in range(n_tt):
                        for ob in range(DL // NB):
                            osl = bass.ts(ob, NB)
                            ps = wps.tile([P, NB], f32, name="ps")
                            for i in range(fb_per_chunk):
                                fb = fb0 + i
                                nc.tensor.matmul(
                                    ps[:], lhsT=af_t[fb][:, bass.ts(tt, P)],
                                    rhs=wo_t[fb][:, bass.ts(ob, NB)],
                                    start=(i == 0), stop=(i == fb_per_chunk - 1))
                            if c == 0:
                                nc.vector.tensor_copy(acc[tt][:, osl], ps[:])
                            elif c < N_CHUNK - 1:
                                nc.vector.tensor_tensor(acc[tt][:, osl],
                                                        acc[tt][:, osl], ps[:],
                                                        op=addop2)
                            else:
                                o = oev.tile([P, NB], f32, tag="o", name="o")
                                nc.vector.tensor_tensor(o[:], acc[tt][:, osl],
                                                        ps[:], op=addop2)
                                nc.sync.dma_start(
                                    out_part[tt * P:(tt + 1) * P, osl], o[:])

    nc.compile()
    return nc


# ---------------- host side ----------------

_ROPE_PERM = np.concatenate([np.arange(0, HD, 2), np.arange(1, HD, 2)])


def host_prep(inputs, t=T):
    """Build per-core input maps from the full problem inputs."""
    x = np.asarray(inputs["x"])[:, :t, :]
    wq, wk, wv, wo = (np.asarray(inputs[k]) for k in ("wq", "wk", "wv", "wo"))
    fcos = np.asarray(inputs["freqs_cos"])[:t]
    fsin = np.asarray(inputs["freqs_sin"])[:t]
    mask = np.asarray(inputs["mask"])

    cosT = np.ascontiguousarray(fcos.T)          # (64, t)
    sinT = np.ascontiguousarray(fsin.T)
    cosh = np.concatenate([cosT, cosT], 0).astype(np.float32)    # (128, t)
    sinh = np.concatenate([-sinT, sinT], 0).astype(np.float32)
    # multiplicative mask tile: exp(mask) on the transposed diagonal block
    # (reference computes softmax(s/sqrt(HD) + mask), and exp(a+m)=exp(a)exp(m))
    with np.errstate(over="ignore"):
        trimulT = np.exp(np.ascontiguousarray(mask[0:P, 0:P].T)).astype(BF16)

    perm = np.concatenate([h * HD + _ROPE_PERM for h in range(HL)])

    # wo input-feature order after chunked gather:
    # chunk c = [g0 heads 2c..2c+1 | g1 heads 2c..2c+1]
    forder = np.empty(2 * DL, np.int64)
    hp = (HL // N_CHUNK) * HD                     # features per group per chunk
    for c in range(N_CHUNK):
        base = c * 2 * hp
        forder[base:base + hp] = np.arange(c * hp, (c + 1) * hp)
        forder[base + hp:base + 2 * hp] = DL + np.arange(c * hp, (c + 1) * hp)

    xTs = [np.ascontiguousarray(x[b].astype(BF16).T) for b in range(B)]
    per_g = []
    for g in range(2):
        sl = slice(g * DL, (g + 1) * DL)
        per_g.append({
            "wqT": np.ascontiguousarray(wq[sl][perm].astype(BF16).T),
            "wkT": np.ascontiguousarray(wk[sl][perm].astype(BF16).T),
            "wvT": np.ascontiguousarray(wv[sl].astype(BF16).T),
            "woT": np.ascontiguousarray(wo[sl][:, forder].astype(BF16).T),
        })

    in_maps = []
    for c in range(N_CORES):
        b, g = c // 2, c % 2
        m = {"xT": xTs[b], "cosh": cosh, "sinh": sinh, "trimulT": trimulT}
        m.update(per_g[g])
        in_maps.append(m)
    return in_maps


_PROGRAM_CACHE = {}


def get_program(t=T, n_cores=N_CORES, with_collective="tail"):
    key = (t, n_cores, with_collective)
    if key not in _PROGRAM_CACHE:
        _PROGRAM_CACHE[key] = build_program(t, n_cores, with_collective)
    return _PROGRAM_CACHE[key]


def assemble(results, t=T):
    out = np.empty((B, t, D), np.float32)
    for c in range(N_CORES):
        b, g = c // 2, c % 2
        out[b, :, g * DL:(g + 1) * DL] = results[c]["out_part"]
    return out


def kernel(**inputs):
    nc = get_program()
    in_maps = host_prep(inputs)
    res = run_bass_kernel_spmd(nc, in_maps, core_ids=list(range(N_CORES)))
    return assemble(res.results)


# revision 11
# speedup vs baseline: 1.0182x; 1.0182x over previous
"""Trainium2 Bass kernel for multi-head causal attention with RoPE.

Problem: B=4, T=2048, D=2048, H=16 heads (HD=128), fp32 reference:
  q/k/v = x @ w{q,k,v}.T ; RoPE(q,k) ; causal softmax(q k^T/sqrt(HD)) @ v ; @ wo.T

Sharding over 8 cores: 4 batch shards x 2 head-groups (8 heads each).
Each core: projections for its heads (column-split weights), attention,
chunked AllGather of attention outputs within the batch pair (overlapped with
attention compute), then output-column-split wo matmul (each core produces
out[:, g*1024:(g+1)*1024] for its batch).

Matmuls run in bf16 (PE at 1 cycle/row); accumulation + softmax in fp32.
Attention computes S^T = k^T q per block so no probs transpose is needed;
softmax row-sums come from a ones-matmul (broadcast across partitions) and
normalization is folded into the PSUM eviction multiply.
"""

import math
import sys
from contextlib import ExitStack

sys.path.insert(0, "/opt/trn_rl_repo")

import numpy as np
import ml_dtypes

import concourse.bass as bass
import concourse.mybir as mybir
import concourse.tile as tile
from concourse import bacc
from concourse.bass_utils import run_bass_kernel_spmd

BF16 = ml_dtypes.bfloat16
B, T, D, H, HD = 4, 2048, 2048, 16, 128
HL = 8            # heads per core
DL = HL * HD      # local feature width (1024)
P = 128
NB = 512          # free-dim block for matmuls
N_CORES = 8
N_CHUNK = 4       # gather chunks (2 heads each)
INV_SQRT_HD = 1.0 / math.sqrt(HD)

dt = mybir.dt
f32 = dt.float32
bf16 = dt.bfloat16


def build_program(t=T, n_cores=N_CORES, with_collective="tail"):
    """Build + compile the per-core Bass program (SPMD, identical on all cores)."""
    n_tb = t // NB      # 512-wide token blocks
    n_tt = t // P       # 128-wide token tiles
    n_db = D // P       # contraction blocks over model dim
    n_q = t // NB       # query blocks (512)

    mult = mybir.AluOpType.mult
    addop = mybir.AluOpType.add
    Exp = mybir.ActivationFunctionType.Exp

    nc = bacc.Bacc("TRN2", target_bir_lowering=False, debug=False,
                   num_devices=n_cores)

    xT = nc.dram_tensor("xT", [D, t], bf16, kind="ExternalInput").ap()
    wqT = nc.dram_tensor("wqT", [D, DL], bf16, kind="ExternalInput").ap()
    wkT = nc.dram_tensor("wkT", [D, DL], bf16, kind="ExternalInput").ap()
    wvT = nc.dram_tensor("wvT", [D, DL], bf16, kind="ExternalInput").ap()
    woT = nc.dram_tensor("woT", [D, DL], bf16, kind="ExternalInput").ap()
    cosh = nc.dram_tensor("cosh", [P, t], f32, kind="ExternalInput").ap()
    sinh = nc.dram_tensor("sinh", [P, t], f32, kind="ExternalInput").ap()
    trimulT = nc.dram_tensor("trimulT", [P, P], bf16, kind="ExternalInput").ap()
    out_part = nc.dram_tensor("out_part", [t, DL], f32, kind="ExternalOutput").ap()

    heads_per_chunk = HL // N_CHUNK

    with tile.TileContext(nc) as tc:
        with tc.tile_pool(name="dram", bufs=1, space="DRAM") as dram:
            attnLc = [dram.tile([heads_per_chunk * P, t], bf16, name=f"attnL{c}")
                      for c in range(N_CHUNK)]
            attnFc = [dram.tile([2 * heads_per_chunk * P, t], bf16,
                                name=f"attnF{c}")
                      for c in range(N_CHUNK)]

            with ExitStack() as es:
                persist = es.enter_context(tc.tile_pool(name="persist", bufs=1))
                xpool_cm = tc.tile_pool(name="xpool", bufs=2)
                xpool = xpool_cm.__enter__()
                qT = [persist.tile([P, t], bf16, tag=f"qT{h}", name=f"qT{h}")
                      for h in range(HL)]
                kT = [persist.tile([P, t], bf16, tag=f"kT{h}", name=f"kT{h}")
                      for h in range(HL)]
                tri_sb = persist.tile([P, P], bf16, tag="tri")
                ones_sb = persist.tile([P, P], bf16, tag="ones")
                nc.vector.memset(ones_sb[:], 1.0)

                # ============ Phase 1a: Q+K projections (share x tiles) ======
                with (
                    tc.tile_pool(name="ropec", bufs=1) as ropec,
                    tc.tile_pool(name="wpool", bufs=1) as wpool,
                    tc.tile_pool(name="pj_psum", bufs=4, space="PSUM") as pjp,
                    tc.tile_pool(name="rope_tmp", bufs=3) as rtmp,
                ):
                    wq_t = [wpool.tile([P, DL], bf16, tag=f"wq{db}",
                                       name=f"wq{db}") for db in range(n_db)]
                    wk_t = [wpool.tile([P, DL], bf16, tag=f"wk{db}",
                                       name=f"wk{db}") for db in range(n_db)]
                    cos_sb = ropec.tile([P, t], f32, tag="cos")
                    sin_sb = ropec.tile([P, t], f32, tag="sin")
                    for db in range(n_db):
                        nc.sync.dma_start(wq_t[db][:], wqT[db * P:(db + 1) * P, :])

                    def rope_evict(ps, dst_tile, tsl):
                        u = rtmp.tile([P, NB], f32, tag="u", name="u")
                        nc.vector.tensor_tensor(u[0:64, :], ps[64:128, :],
                                                sin_sb[0:64, tsl], op=mult)
                        nc.vector.tensor_tensor(u[64:128, :], ps[0:64, :],
                                                sin_sb[64:128, tsl], op=mult)
                        nc.vector.tensor_tensor(dst_tile, ps[:], cos_sb[:, tsl],
                                                op=mult)
                        nc.vector.tensor_tensor(dst_tile, dst_tile, u[:], op=addop)

                    for tb in range(n_tb):
                        tsl = bass.ts(tb, NB)
                        x_t = [xpool.tile([P, NB], bf16, tag=f"x{db}",
                                          name=f"x{db}") for db in range(n_db)]
                        for db in range(n_db):
                            nc.sync.dma_start(x_t[db][:],
                                              xT[db * P:(db + 1) * P, tsl])
                        if tb == 0:
                            # emit late-needed loads behind the critical path
                            for db in range(n_db):
                                nc.sync.dma_start(wk_t[db][:],
                                                  wkT[db * P:(db + 1) * P, :])
                            nc.sync.dma_start(cos_sb[:], cosh[:])
                            nc.sync.dma_start(sin_sb[:], sinh[:])
                            nc.sync.dma_start(tri_sb[:], trimulT[:])
                        for jt in range(HL):
                            for (w_t, dst) in ((wq_t, qT), (wk_t, kT)):
                                ps = pjp.tile([P, NB], f32, name="ps")
                                for db in range(n_db):
                                    nc.tensor.matmul(
                                        ps[:], lhsT=w_t[db][:, bass.ts(jt, P)],
                                        rhs=x_t[db][:],
                                        start=(db == 0), stop=(db == n_db - 1))
                                rope_evict(ps, dst[jt][:, tsl], tsl)

                # ============ Phase 1b: V projection =========================
                # v tiles live in their own pool, reusing the closed wpool space
                vpool = es.enter_context(
                    tc.tile_pool(name="vpool", bufs=1, side="right"))
                v = [vpool.tile([P, DL], bf16, tag=f"v{tt}", name=f"v{tt}")
                     for tt in range(n_tt)]
                with (
                    tc.tile_pool(name="wvpool", bufs=1) as wvpool,
                    tc.tile_pool(name="v_psum", bufs=4, space="PSUM") as vps,
                ):
                    wv_t = [wvpool.tile([P, DL], bf16, tag=f"wv{db}",
                                        name=f"wv{db}") for db in range(n_db)]
                    for db in range(n_db):
                        nc.sync.dma_start(wv_t[db][:], wvT[db * P:(db + 1) * P, :])
                    for tb in range(n_tb):
                        x_t = [xpool.tile([P, NB], bf16, tag=f"x{db}",
                                          name=f"x{db}") for db in range(n_db)]
                        for db in range(n_db):
                            nc.sync.dma_start(
                                x_t[db][:], xT[db * P:(db + 1) * P, bass.ts(tb, NB)])
                        for tq in range(4):
                            tt = tb * 4 + tq
                            for jb in range(DL // NB):
                                ps = vps.tile([P, NB], f32, name="ps")
                                for db in range(n_db):
                                    nc.tensor.matmul(
                                        ps[:], lhsT=x_t[db][:, bass.ts(tq, P)],
                                        rhs=wv_t[db][:, bass.ts(jb, NB)],
                                        start=(db == 0), stop=(db == n_db - 1))
                                nc.vector.tensor_copy(v[tt][:, bass.ts(jb, NB)],
                                                      ps[:])
                xpool_cm.__exit__(None, None, None)

                # ============ Phase 2: attention (+ chunked gather) ==========
                with (
                    tc.tile_pool(name="st_psum", bufs=3, space="PSUM") as stp,
                    tc.tile_pool(name="ot_psum", bufs=2, space="PSUM") as otp,
                    tc.tile_pool(name="sum_psum", bufs=2, space="PSUM") as smp,
                    tc.tile_pool(name="ptu", bufs=6) as ptup,
                    tc.tile_pool(name="att_ev", bufs=4) as atev,
                ):
                    for h in range(HL):
                        for q in range(n_q):
                            qsl = bass.ts(q, NB)
                            nkb = 4 * q + 4
                            ot = otp.tile([P, NB], f32, tag="ot", name="ot")
                            sums = smp.tile([P, NB], f32, tag="sums", name="sums")
                            for kb in range(nkb):
                                rel = kb - 4 * q
                                st = stp.tile([P, NB], f32, tag="st", name="st")
                                nc.tensor.matmul(
                                    st[:], lhsT=kT[h][:, bass.ts(kb, P)],
                                    rhs=qT[h][:, qsl], start=True, stop=True)
                                ptu = ptup.tile([P, NB], bf16, tag="ptu",
                                                name="ptu")
                                if rel >= 0:
                                    if rel > 0:
                                        nc.gpsimd.memset(ptu[:, 0:rel * P], 0.0)
                                    esl = bass.ds(rel * P, NB - rel * P)
                                    nc.scalar.activation(ptu[:, esl], st[:, esl],
                                                         Exp, scale=INV_SQRT_HD)
                                    csl = bass.ds(rel * P, P)
                                    nc.vector.tensor_tensor(
                                        ptu[:, csl], ptu[:, csl], tri_sb[:],
                                        op=mult)
                                else:
                                    nc.scalar.activation(ptu[:], st[:],
                                                         Exp, scale=INV_SQRT_HD)
                                nc.tensor.matmul(
                                    sums[:], lhsT=ones_sb[:], rhs=ptu[:],
                                    start=(kb == 0), stop=(kb == nkb - 1))
                                nc.tensor.matmul(
                                    ot[:], lhsT=v[kb][:, bass.ts(h, P)], rhs=ptu[:],
                                    start=(kb == 0), stop=(kb == nkb - 1))
                            # normalize with 1/sums (already partition-broadcast)
                            rb = atev.tile([P, NB], f32, tag="rb", name="rb")
                            nc.vector.reciprocal_approx_fast(out=rb[:], in_=sums[:])
                            att = atev.tile([P, NB], bf16, tag="att", name="att")
                            nc.vector.tensor_tensor(att[:], ot[:], rb[:], op=mult)
                            c = h // heads_per_chunk
                            row = (h % heads_per_chunk) * P
                            nc.sync.dma_start(attnLc[c][row:row + P, qsl], att[:])
                        if (with_collective == "inline"
                                and h % heads_per_chunk == heads_per_chunk - 1):
                            c = h // heads_per_chunk
                            nc.gpsimd.collective_compute(
                                "AllGather", mybir.AluOpType.bypass,
                                replica_groups=[[i, i + 1]
                                                for i in range(0, n_cores, 2)],
                                ins=[attnLc[c].opt()], outs=[attnFc[c].opt()],
                            )

            if with_collective == "tail":
                for c in range(N_CHUNK):
                    nc.gpsimd.collective_compute(
                        "AllGather", mybir.AluOpType.bypass,
                        replica_groups=[[i, i + 1]
                                        for i in range(0, n_cores, 2)],
                        ins=[attnLc[c].opt()], outs=[attnFc[c].opt()],
                    )
            elif not with_collective:
                for c in range(N_CHUNK):
                    hp = heads_per_chunk * P
                    nc.sync.dma_start(attnFc[c][0:hp, :], attnLc[c][:])
                    nc.sync.dma_start(attnFc[c][hp:2 * hp, :], attnLc[c][:])

            # ============ Phase 3: wo matmul =============================
            n_fb = (2 * DL) // P
            fb_per_chunk = n_fb // N_CHUNK
            with (
                tc.tile_pool(name="afpool", bufs=1) as afp,
                tc.tile_pool(name="wopool", bufs=1) as wop,
                tc.tile_pool(name="wo_psum", bufs=4, space="PSUM") as wps,
                tc.tile_pool(name="out_ev", bufs=4) as oev,
            ):
                af_t = [afp.tile([P, t], bf16, tag=f"af{fb}", name=f"af{fb}")
                        for fb in range(n_fb)]
                for fb in range(n_fb):
                    c, r = fb // fb_per_chunk, fb % fb_per_chunk
                    nc.sync.dma_start(af_t[fb][:],
                                      attnFc[c][r * P:(r + 1) * P, :])
                wo_t = [wop.tile([P, DL], bf16, tag=f"wo{fb}", name=f"wo{fb}")
                        for fb in range(n_fb)]
                for fb in range(n_fb):
                    nc.sync.dma_start(wo_t[fb][:], woT[fb * P:(fb + 1) * P, :])
                for tt in range(n_tt):
                    for ob in range(DL // NB):
                        ps = wps.tile([P, NB], f32, name="ps")
                        for fb in range(n_fb):
                            nc.tensor.matmul(
                                ps[:], lhsT=af_t[fb][:, bass.ts(tt, P)],
                                rhs=wo_t[fb][:, bass.ts(ob, NB)],
                                start=(fb == 0), stop=(fb == n_fb - 1))
                        o = oev.tile([P, NB], f32, tag="o", name="o")
                        nc.scalar.copy(o[:], ps[:])
                        nc.sync.dma_start(
                            out_part[tt * P:(tt + 1) * P, bass.ts(ob, NB)], o[:])

    nc.compile()
    return nc


# ---------------- host side ----------------

_ROPE_PERM = np.concatenate([np.arange(0, HD, 2), np.arange(1, HD, 2)])


def host_prep(inputs, t=T):
    """Build per-core input maps from the full problem inputs."""
    x = np.asarray(inputs["x"])[:, :t, :]
    wq, wk, wv, wo = (np.asarray(inputs[k]) for k in ("wq", "wk", "wv", "wo"))
    fcos = np.asarray(inputs["freqs_cos"])[:t]
    fsin = np.asarray(inputs["freqs_sin"])[:t]
    mask = np.asarray(inputs["mask"])

    cosT = np.ascontiguousarray(fcos.T)          # (64, t)
    sinT = np.ascontiguousarray(fsin.T)
    cosh = np.concatenate([cosT, cosT], 0).astype(np.float32)    # (128, t)
    sinh = np.concatenate([-sinT, sinT], 0).astype(np.float32)
    # multiplicative mask tile: exp(mask) on the transposed diagonal block
    # (reference computes softmax(s/sqrt(HD) + mask), and exp(a+m)=exp(a)exp(m))
    with np.errstate(over="ignore"):
        trimulT = np.exp(np.ascontiguousarray(mask[0:P, 0:P].T)).astype(BF16)

    perm = np.concatenate([h * HD + _ROPE_PERM for h in range(HL)])

    # wo input-feature order after chunked gather:
    # chunk c = [g0 heads 2c..2c+1 | g1 heads 2c..2c+1]
    forder = np.empty(2 * DL, np.int64)
    hp = (HL // N_CHUNK) * HD                     # features per group per chunk
    for c in range(N_CHUNK):
        base = c * 2 * hp
        forder[base:base + hp] = np.arange(c * hp, (c + 1) * hp)
        forder[base + hp:base + 2 * hp] = DL + np.arange(c * hp, (c + 1) * hp)

    xTs = [np.ascontiguousarray(x[b].astype(BF16).T) for b in range(B)]
    per_g = []
    for g in range(2):
        sl = slice(g * DL, (g + 1) * DL)
        per_g.append({
            "wqT": np.ascontiguousarray(wq[sl][perm].astype(BF16).T),
            "wkT": np.ascontiguousarray(wk[sl][perm].astype(BF16).T),
            "wvT": np.ascontiguousarray(wv[sl].astype(BF16).T),
            "woT": np.ascontiguousarray(wo[sl][:, forder].astype(BF16).T),
        })

    in_maps = []
    for c in range(N_CORES):
        b, g = c // 2, c % 2
        m = {"xT": xTs[b], "cosh": cosh, "sinh": sinh, "trimulT": trimulT}
        m.update(per_g[g])
        in_maps.append(m)
    return in_maps


_PROGRAM_CACHE = {}


def get_program(t=T, n_cores=N_CORES, with_collective="tail"):
    key = (t, n_cores, with_collective)
    if key not in _PROGRAM_CACHE:
        _PROGRAM_CACHE[key] = build_program(t, n_cores, with_collective)
    return _PROGRAM_CACHE[key]


def assemble(results, t=T):
    out = np.empty((B, t, D), np.float32)
    for c in range(N_CORES):
        b, g = c // 2, c % 2
        out[b, :, g * DL:(g + 1) * DL] = results[c]["out_part"]
    return out


def kernel(**inputs):
    nc = get_program()
    in_maps = host_prep(inputs)
    res = run_bass_kernel_spmd(nc, in_maps, core_ids=list(range(N_CORES)))
    return assemble(res.results)
